# revision 1
# baseline (speedup 1.0000x reference)
"""LocallyConnected2d Trainium2 kernel.

Problem: out[b,o,oh,ow] = sum_{c,ki,kj} x[b,c,oh+ki,ow+kj] * W[o,oh,ow,c,ki,kj] + bias[o,oh,ow]
Shapes: x[32,32,64,64], W[64,62,62,32,3,3], bias[64,62,62] -> out[32,64,62,62], all fp32.

Strategy (8 NeuronCores, sharded over output rows, 8 rows/core padded to 64):
- Per output location: 3 accumulating PE matmuls, K=97 each (chunk q = kernel
  row ki; features j=(kj,c) plus a ones-row at j=96 that carries bias on q=2).
- lhsT (stationary) = x patch columns [97,32b]: x is loaded into SBUF once as
  3 column-shifted replicas on partitions kj*32+c, so every lhsT is a direct
  AP slice (no im2col data movement). Partition 96 = constant 1.0.
- rhs (moving) = per-location weights [97,64o], streamed from HBM in
  half-row strips with a host-side layout [row, j, q, ow, o] that makes each
  DMA fully contiguous per partition.
- PSUM accumulates [32b, 64o] per location, 8 locations per bank; DVE copies
  each group to an SBUF out strip; one contiguous DMA per half-row out.
"""

import numpy as np

import concourse.bass as bass  # noqa: F401
import concourse.mybir as mybir
import concourse.tile as tile
from concourse import bacc
from concourse.bass_utils import run_bass_kernel_spmd

B, C_IN, H, W = 32, 32, 64, 64
C_OUT, OH, OW, KK = 64, 62, 62, 3
N_CORES = 8
ROWS = 8          # padded output rows per core (8*8=64 >= 62)
HALF = 31         # locations per strip (half an output row)
XH = ROWS + 2     # input rows needed per core
KP = 97           # contraction per chunk: 96 features + ones/bias row
F32 = mybir.dt.float32

_NC_CACHE = {}


def _build_nc():
    nc = bacc.Bacc(
        "TRN2",
        target_bir_lowering=False,
        debug=False,
        enable_asserts=False,
        num_devices=N_CORES,
    )
    # x ships host-transposed AND pre-shifted into 3 kj-replicas
    # [kj, c, h, w(62), b] so the whole x3 load is one contiguous DMA
    x_d = nc.dram_tensor("x", [KK, C_IN, XH, OW, B], F32, kind="ExternalInput").ap()
    # w ships pre-split by half-row strip: [row, half, j, q, l, o] so each
    # strip DMA is one fully-contiguous block (97 x 23.8KB descriptors)
    w_d = nc.dram_tensor(
        "w", [ROWS, 2, KP, 3, HALF, C_OUT], F32, kind="ExternalInput"
    ).ap()
    ones_d = nc.dram_tensor("ones", [1, XH * OW * B], F32, kind="ExternalInput").ap()
    # out layout: [row, half, p=(l4,b), grp, o] - 4 locations (col groups)
    # stacked on PSUM/SBUF partitions; host unscrambles
    NG = 8  # ceil(31/4) location groups per strip
    o_d = nc.dram_tensor(
        "out", [ROWS, 2, 128, NG, C_OUT], F32, kind="ExternalOutput"
    ).ap()

    with tile.TileContext(nc) as tc:
        with (
            tc.tile_pool(name="xpool", bufs=1) as xpool,
            tc.tile_pool(name="wpool", bufs=5) as wpool,
            tc.tile_pool(name="opool", bufs=2) as opool,
            tc.tile_pool(name="pspool", bufs=8, space="PSUM") as pspool,
        ):
            # x replicas: partition kj*32+c holds x[b,c,h,w+kj] at free
            # (h, w, b); partition 96 = 1.0 (carries the bias row).
            # Contiguous layout -> large (39.7KB) DMA descriptors; throughput
            # comes from multiple concurrent sub-DMAs (each in-flight
            # InstDMACopy has its own outstanding-descriptor window).
            HZ = OW * B  # 1984
            x3 = xpool.tile([KP, XH * HZ], F32)
            nc.sync.dma_start(out=x3[96:97, :], in_=ones_d)
            xsrc = x_d.rearrange("k c h w b -> (k c) (h w b)")

            def load_x_rows(r0, r1, eng=None):
                for p0, p1 in ((0, 32), (32, 64), (64, 96)):
                    (eng or nc.gpsimd).dma_start(
                        out=x3[p0:p1, r0 * HZ : r1 * HZ],
                        in_=xsrc[p0:p1, r0 * HZ : r1 * HZ],
                    )

            # rows 0-2 up front (first output row); rows 4-7 ride the
            # otherwise-idle sync HWDGE ring; rows 3, 8, 9 interleave below
            load_x_rows(0, 3)
            load_x_rows(4, 8, eng=nc.sync)

            QZ = HALF * C_OUT  # 1984, one chunk per kernel row q
            XROW_PREFETCH = {(0, 1): (3, 4), (2, 0): (8, 9), (3, 0): (9, 10)}
            for row in range(ROWS):
                for half in range(2):
                    if (row, half) in XROW_PREFETCH:
                        load_x_rows(*XROW_PREFETCH[(row, half)])
                    strip = row * 2 + half
                    wt = wpool.tile([KP, 3 * QZ], F32, tag="wt")
                    # 3 sub-DMAs by partition range -> 3 concurrent windows,
                    # each with one 23.8KB contiguous descriptor per partition.
                    # First/last strips split additionally by q-chunk so the
                    # first q=0 matmuls unblock after 1/3 of the strip.
                    wsrc = w_d[row, half].rearrange("p q l o -> p (q l o)")
                    if strip == 15:
                        # finer split so the last q=0 matmuls unblock early
                        for f0, f1 in ((0, QZ), (QZ, 2 * QZ), (2 * QZ, 3 * QZ)):
                            for p0, p1 in ((0, 32), (32, 64), (64, KP)):
                                nc.gpsimd.dma_start(
                                    out=wt[p0:p1, f0:f1], in_=wsrc[p0:p1, f0:f1]
                                )
                    else:
                        for p0, p1 in ((0, 32), (32, 64), (64, KP)):
                            nc.gpsimd.dma_start(out=wt[p0:p1, :], in_=wsrc[p0:p1])
                    ot = opool.tile([128, NG * C_OUT], F32, tag="ot")
                    otv = ot.rearrange("p (g o) -> p g o", g=NG, o=C_OUT)
                    for g in range(NG):
                        gn = min(4, HALF - g * 4)  # 4,4,...,3
                        # 4 locations packed into PE col groups: out slice
                        # base partition 32*l selects the col group, so the
                        # 4 locations' matmuls can overlap in the array
                        ps = pspool.tile([128, C_OUT], F32, tag="ps")
                        for li in range(4):
                            # pad slot in the last group duplicates the prior
                            # location (keeps PSUM fully written; host drops it)
                            eff = min(li, gn - 1)
                            ow = half * HALF + g * 4 + eff
                            for q in range(3):
                                loff = (g * 4 + eff) * C_OUT
                                nc.tensor.matmul(
                                    ps[32 * li : 32 * li + 32, :],
                                    x3[
                                        :,
                                        (row + q) * HZ
                                        + ow * B : (row + q) * HZ
                                        + ow * B
                                        + B,
                                    ],  # [97, 32] lhsT
                                    wt[:, q * QZ + loff : q * QZ + loff + C_OUT],
                                    start=(q == 0),
                                    stop=(q == 2),
                                    tile_position=(0, 32 * li),
                                )
                        nc.vector.tensor_copy(out=otv[:, g, :], in_=ps)
                    # scalar = second HWDGE ring: keeps out-stores off the
                    # gpsimd FIFO so w prefetch is never head-of-line blocked.
                    # Last two strips go via gpsimd (idle by then, and SWDGE
                    # is much faster) to shrink the tail.
                    oeng = nc.gpsimd if strip >= 14 else nc.scalar
                    oeng.dma_start(out=o_d[row, half], in_=ot)

    nc.compile()
    return nc


def get_nc():
    if "nc" not in _NC_CACHE:
        _NC_CACHE["nc"] = _build_nc()
    return _NC_CACHE["nc"]


def prep_inputs(x, weight, bias):
    """Host-side shard + layout prep. Returns per-core in_maps."""
    x = np.asarray(x, dtype=np.float32)
    weight = np.asarray(weight, dtype=np.float32)
    bias = np.asarray(bias, dtype=np.float32)

    # w_prep[oh, j=kj*32+c, q=ki, ow, o]; j=96 row: 0 for q<2, bias for q=2
    wp = np.zeros((N_CORES * ROWS, KP, 3, OW, C_OUT), np.float32)
    wp[:OH, :96] = weight.transpose(1, 5, 3, 4, 2, 0).reshape(OH, 96, 3, OW, C_OUT)
    wp[:OH, 96, 2] = bias.transpose(1, 2, 0)
    # split ow into half-row strips: [row, half, j, q, l, o]
    wp = np.ascontiguousarray(
        wp.reshape(N_CORES * ROWS, KP, 3, 2, HALF, C_OUT).transpose(0, 3, 1, 2, 4, 5)
    )

    xp = np.zeros((B, C_IN, N_CORES * ROWS + 2, W), np.float32)
    xp[:, :, :H] = x
    xt = xp.transpose(1, 2, 3, 0)  # [c, h, w, b]

    ones = np.ones((1, XH * OW * B), np.float32)

    in_maps = []
    for c in range(N_CORES):
        r0 = c * ROWS
        xc = xt[:, r0 : r0 + XH]  # [c, 10, 64, b]
        xsh = np.stack([xc[:, :, kj : kj + OW, :] for kj in range(KK)])
        in_maps.append(
            {
                "x": np.ascontiguousarray(xsh),
                "w": np.ascontiguousarray(wp[r0 : r0 + ROWS]),
                "ones": ones,
            }
        )
    return in_maps


def gather_output(results):
    """results: list of per-core out dicts -> full [B, C_OUT, OH, OW]."""
    out = np.empty((B, C_OUT, OH, OW), np.float32)
    for c in range(N_CORES):
        oc = results[c]["out"]  # [ROWS, 2, 128=(l4,b), NG, C_OUT]
        v = oc.reshape(ROWS, 2, 4, B, 8, C_OUT)
        # ow = half*31 + grp*4 + l  (grp*4+l < 31)
        arr = v.transpose(3, 5, 0, 1, 4, 2).reshape(B, C_OUT, ROWS, 2, 32)
        arr = arr[:, :, :, :, :HALF].reshape(B, C_OUT, ROWS, OW)
        r0 = c * ROWS
        rows = min(ROWS, OH - r0)
        out[:, :, r0 : r0 + rows, :] = arr[:, :, :rows, :]
    return out


def run(inputs, **kw):
    nc = get_nc()
    in_maps = prep_inputs(inputs["x"], inputs["weight"], inputs["bias"])
    res = run_bass_kernel_spmd(nc, in_maps, core_ids=list(range(N_CORES)), **kw)
    return gather_output(res.results), res


def kernel(x, weight, bias):
    out, _ = run({"x": x, "weight": weight, "bias": bias})
    return out



# revision 7
# speedup vs baseline: 1.9368x; 1.9368x over previous
"""LocallyConnected2d Trainium2 kernel (bf16).

Problem: out[b,o,oh,ow] = sum_{c,ki,kj} x[b,c,oh+ki,ow+kj] * W[o,oh,ow,c,ki,kj] + bias[o,oh,ow]
Shapes: x[32,32,64,64], W[64,62,62,32,3,3], bias[64,62,62] -> out[32,64,62,62], fp32 in/out.

The kernel is DMA-bandwidth-bound on the weight stream (every weight is used
once per batch element, 16 FLOP/byte at fp32), so everything is shipped and
computed in bf16 (l2 rel err ~2e-3, budget 2e-2):
- Per output location: 3 accumulating PE matmuls into fp32 PSUM, K=97 each
  (chunk q = kernel row ki; features j=(kj,c) plus a ones-row at j=96 that
  carries bias on q=2).
- lhsT (stationary) = x patch columns [97,32b]: x is loaded into SBUF once as
  3 column-shifted replicas on partitions kj*32+c, so every lhsT is a direct
  AP slice (no im2col data movement). Partition 96 = memset 1.0.
- rhs (moving) = per-location weights [97,64o], streamed from HBM one output
  row at a time (2.3 MB per row, split into 3 q-chunk dma_starts so the q=0
  matmuls unblock after 1/3 of a row).
- PSUM accumulates [32b, 64o] per location; 4 locations stacked on PSUM
  partitions (PE column groups) x 8 groups = one full bank [128,512] per 32
  locations; one DVE copy per bank converts to a bf16 out strip; one
  contiguous 256KB DMA per output row.
"""

import numpy as np
from ml_dtypes import bfloat16

import concourse.bass as bass  # noqa: F401
import concourse.mybir as mybir
import concourse.tile as tile
from concourse import bacc
from concourse.bass_utils import run_bass_kernel_spmd

B, C_IN, H, W = 32, 32, 64, 64
C_OUT, OH, OW, KK = 64, 62, 62, 3
N_CORES = 8
ROWS = 8          # padded output rows per core (8*8=64 >= 62)
XH = ROWS + 2     # input rows needed per core
KP = 97           # contraction per chunk: 96 features + ones/bias row
HZ = OW * B       # 1984 x3 free elems per input row
QZ = OW * C_OUT   # 3968 w free elems per q chunk
F32 = mybir.dt.float32
BF16 = mybir.dt.bfloat16

_NC_CACHE = {}


def _build_nc():
    nc = bacc.Bacc(
        "TRN2",
        target_bir_lowering=False,
        debug=False,
        enable_asserts=False,
        num_devices=N_CORES,
    )
    # x ships host-transposed AND pre-shifted into 3 kj-replicas
    # [kj, c, h, w(62), b] so the whole x3 load is contiguous DMA
    x_d = nc.dram_tensor("x", [KK, C_IN, XH, OW, B], BF16, kind="ExternalInput").ap()
    # w ships pre-split by output row: [row, j, q, ow, o] so each q-chunk
    # DMA is one fully-contiguous 7.9KB descriptor per partition
    w_d = nc.dram_tensor("w", [ROWS, KP, 3, OW, C_OUT], BF16, kind="ExternalInput").ap()
    # out layout: [row, p=(l4,b), grp, o] - 4 locations (col groups)
    # stacked on PSUM/SBUF partitions; host unscrambles
    NG = 16  # ceil(62/4) location groups per row
    o_d = nc.dram_tensor("out", [ROWS, 128, NG * C_OUT], BF16, kind="ExternalOutput").ap()

    with tile.TileContext(nc) as tc:
        with (
            tc.tile_pool(name="xpool", bufs=1) as xpool,
            tc.tile_pool(name="wpool", bufs=3) as wpool,
            tc.tile_pool(name="opool", bufs=2) as opool,
            tc.tile_pool(name="pspool", bufs=8, space="PSUM") as pspool,
        ):
            # x replicas: partition kj*32+c holds x[b,c,h,w+kj] at free
            # (h, w, b); partition 96 = 1.0 (carries the bias row).
            x3 = xpool.tile([KP, XH * HZ], BF16)
            nc.vector.memset(x3[96:97, :], 1.0)
            xsrc = x_d.rearrange("k c h w b -> (k c) (h w b)")

            def load_x_rows(r0, r1, eng):
                eng.dma_start(
                    out=x3[0:96, r0 * HZ : r1 * HZ],
                    in_=xsrc[:, r0 * HZ : r1 * HZ],
                )

            # rows 0-2 up front (first output row); the rest ride behind on
            # the same sync HWDGE ring, arriving during rows 0-2 compute
            load_x_rows(0, 3, nc.sync)
            load_x_rows(3, 6, nc.sync)
            load_x_rows(6, 10, nc.sync)

            for row in range(ROWS):
                wt = wpool.tile([KP, 3 * QZ], BF16, tag="wt")
                # 3 sub-DMAs by partition range -> 3 concurrent windows, each
                # with one 23.8KB contiguous descriptor per partition. (Not
                # split by q chunk: staggered q arrival makes the scheduler
                # defer q=2 stop-matmuls, piling up open PSUM groups.)
                wsrc = w_d[row].rearrange("p q l o -> p (q l o)")
                for p0, p1 in ((0, 32), (32, 64), (64, KP)):
                    nc.gpsimd.dma_start(out=wt[p0:p1, :], in_=wsrc[p0:p1, :])
                ot = opool.tile([128, NG * C_OUT], BF16, tag="ot")
                otv = ot.rearrange("p (g o) -> p g o", g=NG, o=C_OUT)
                for g in range(NG):
                    gn = min(4, OW - g * 4)  # 4,...,4,2
                    # 4 locations packed into PE col groups: out slice
                    # base partition 32*l selects the col group, so the
                    # 4 locations' matmuls can overlap in the array
                    ps = pspool.tile([128, C_OUT], F32, tag="ps")
                    for li in range(4):
                        # pad slot in the last group duplicates the prior
                        # location (keeps PSUM fully written; host drops it).
                        # li outer / q inner: each location's start..stop
                        # accumulation nests fully before the next starts
                        # (the sim's zero-region tracking aliases partition-
                        # sliced PSUM offsets, so interleaved starts trip it).
                        eff = min(li, gn - 1)
                        ow = g * 4 + eff
                        for q in range(3):
                            nc.tensor.matmul(
                                ps[32 * li : 32 * li + 32, :],
                                x3[:, (row + q) * HZ + ow * B : (row + q) * HZ + ow * B + B],
                                wt[:, q * QZ + ow * C_OUT : q * QZ + ow * C_OUT + C_OUT],
                                start=(q == 0),
                                stop=(q == 2),
                                tile_position=(0, 32 * li),
                            )
                    nc.vector.tensor_copy(out=otv[:, g, :], in_=ps)
                # scalar = second HWDGE ring: keeps out-stores off the
                # gpsimd FIFO so w prefetch is never head-of-line blocked.
                # Last row goes via gpsimd (idle by then) to shrink the tail.
                oeng = nc.gpsimd if row == ROWS - 1 else nc.scalar
                oeng.dma_start(out=o_d[row], in_=ot)

    nc.compile()
    return nc


def get_nc():
    if "nc" not in _NC_CACHE:
        _NC_CACHE["nc"] = _build_nc()
    return _NC_CACHE["nc"]


def prep_inputs(x, weight, bias):
    """Host-side shard + layout prep. Returns per-core in_maps."""
    x = np.asarray(x, dtype=np.float32)
    weight = np.asarray(weight, dtype=np.float32)
    bias = np.asarray(bias, dtype=np.float32)

    # w_prep[oh, j=kj*32+c, q=ki, ow, o]; j=96 row: 0 for q<2, bias for q=2
    wp = np.zeros((N_CORES * ROWS, KP, 3, OW, C_OUT), np.float32)
    wp[:OH, :96] = weight.transpose(1, 5, 3, 4, 2, 0).reshape(OH, 96, 3, OW, C_OUT)
    wp[:OH, 96, 2] = bias.transpose(1, 2, 0)
    wp = wp.astype(bfloat16)

    xp = np.zeros((B, C_IN, N_CORES * ROWS + 2, W), np.float32)
    xp[:, :, :H] = x
    xt = xp.transpose(1, 2, 3, 0).astype(bfloat16)  # [c, h, w, b]

    in_maps = []
    for c in range(N_CORES):
        r0 = c * ROWS
        xc = xt[:, r0 : r0 + XH]  # [c, 10, 64, b]
        xsh = np.stack([xc[:, :, kj : kj + OW, :] for kj in range(KK)])
        in_maps.append(
            {
                "x": np.ascontiguousarray(xsh),
                "w": np.ascontiguousarray(wp[r0 : r0 + ROWS]),
            }
        )
    return in_maps


def gather_output(results):
    """results: list of per-core out dicts -> full [B, C_OUT, OH, OW]."""
    out = np.empty((B, C_OUT, OH, OW), np.float32)
    for c in range(N_CORES):
        oc = np.asarray(results[c]["out"], dtype=np.float32)  # [ROWS, 128, 16*C_OUT]
        v = oc.reshape(ROWS, 4, B, 16, C_OUT)  # [r, l, b, g, o]
        # ow = 4*g + l  (only ow < 62 valid)
        arr = v.transpose(2, 4, 0, 3, 1).reshape(B, C_OUT, ROWS, 64)[:, :, :, :OW]
        r0 = c * ROWS
        rows = min(ROWS, OH - r0)
        out[:, :, r0 : r0 + rows, :] = arr[:, :, :rows, :]
    return out


def run(inputs, **kw):
    nc = get_nc()
    in_maps = prep_inputs(inputs["x"], inputs["weight"], inputs["bias"])
    res = run_bass_kernel_spmd(nc, in_maps, core_ids=list(range(N_CORES)), **kw)
    return gather_output(res.results), res


def kernel(x, weight, bias):
    out, _ = run({"x": x, "weight": weight, "bias": bias})
    return out
